# revision 26
# baseline (speedup 1.0000x reference)
"""FAPE loss Trainium2 kernel.

Math: for frames f (built from coord triples) and points n,
  d2[f,n] = ||Rp(p_n - po_f)||^2 + ||Rt(t_n - to_f)||^2 - 2 (p_n-po_f)^T M (t_n-to_f)
with M = Rp^T Rt.  Expanding, d2[f,n] = X[n] . Y[f] with 17 features:
  X = [A_n, 1, p (3), t (3), W (9)]   A_n = ||p_n||^2 + ||t_n||^2, W = outer(p_n, t_n)
  Y = [mask, B_f - 2c_f + off, 2(u-po), 2(v-to), -2M]  u = M to, v = M^T po,
      c_f = po.u, B_f = ||po||^2 + ||to||^2
Loss = mean(min(sqrt(d2 + eps), 10)) / 10.

The O(N) feature prep (X per point, Y per frame) is done host-side in numpy
and shipped pre-transposed in the exact matmul layouts, so the device does
only the O(F*N) part: 32 fp32r matmuls (K=17), ACT sqrt, DVE clamped
accumulation, and a scalar reduce.

Sharding: frames split across 8 cores (512/core; the last core's 2 pad
frames have all-zero Y rows).  Points replicated.

Device layout per core:
  xt [96, 1408] f32r: X^T in 11 windows of 128 cols (=128 points); window
      b, slot s in {0..2} holds feature k at partition 32s+k for point
      group g = 3b + s (points g*128 ..); 33rd group slot zero.
  yt [96, 512] f32r: Y^T replicated at partition bases 0/32/64 so every
      lhsT slot finds a matching rhs.
  8 supertiles u of 4 matmuls g = 4u+h (window g//3, slot g%3) -> PSUM
      [128, 2048] f32 -> ACT sqrt(+eps) -> bf16 SBUF s.
      Finish on DVE in fast 16-bit mode: tmp = min(s, 10); acc += tmp
      (min also squashes any NaN from f32r noise on near-zero d2).
      Tail: DMA acc [128, 2048] bf16 straight out; the host sums it
      (the on-device reduce chain cost ~3us serial).
"""
import sys

for _p in ("/opt/trn_rl_repo", "/root/.axon_site/_ro/trn_rl_repo"):
    if _p not in sys.path:
        sys.path.append(_p)

import numpy as np
from concourse import bass, bacc, mybir, tile
from concourse.bass_utils import run_bass_kernel_spmd

F32 = mybir.dt.float32
F32R = mybir.dt.float32r
BF16 = mybir.dt.bfloat16
F16 = mybir.dt.float16
AF = mybir.ActivationFunctionType
OP = mybir.AluOpType

N = 4096          # points
F = N - 2         # frames (4094)
NCORES = 8
FPC = 512         # frames per core (last core: 510 real + 2 zero-pad)
KF = 17           # contraction features
EPS = 1e-8
UNIT = 10.0
CLAMP = 10.0
DSQ_OFF = 1.0     # added to every real frame's d2 so f32r noise can't push
                  # it negative (sqrt(neg)=NaN); ~3.9e-4 relative loss bias
NWIN = 11         # X^T windows of 128 points, 3 feature-slots each
STWIDTHS = [1024] + [2048] * 7 + [1024]   # tapered supertiles (32 matmuls)


def build_nc():
    nc = bacc.Bacc(None)

    # yt (cols 0:512) and xt (cols 512:1920) packed in one input tensor so
    # the sync queue issues fewer DMAs before the block-entry barrier
    xy_d = nc.dram_tensor("xy", [96, 1920], F32R, kind="ExternalInput")
    out_d = nc.dram_tensor("out", [128, 2048], BF16, kind="ExternalOutput")

    with tile.TileContext(nc) as tc:
        with (
            tc.tile_pool(name="inp", bufs=1) as inp,
            tc.tile_pool(name="sp", bufs=3) as sp,
            tc.tile_pool(name="accp", bufs=1) as accp,
            tc.tile_pool(name="psD", bufs=2, space="PSUM") as psD,
        ):
            xy_sb = inp.tile([96, 1920], F32R)
            yt_sb = xy_sb[:, 0:FPC]
            xt_sb = xy_sb[:, FPC:1920]
            # staged input DMAs on the sync HWDGE queue: the first carries
            # only what supertile 0 needs (yt + window 0), so the first
            # matmul fires as early as possible; later windows pipeline in
            # behind it
            nc.sync.dma_start(xy_sb[:, 0:640], xy_d[:, 0:640])
            nc.sync.dma_start(xy_sb[:, 640:1280], xy_d[:, 640:1280])

            epst = inp.tile([128, 1], F32)
            nc.vector.memset(epst[:], EPS)

            acc = accp.tile([128, 2048], BF16)
            nc.vector.memset(acc[:], 0.0)

            g = 0
            for u, w in enumerate(STWIDTHS):
                nmm = w // FPC
                ps = psD.tile([128, 2048], F32, tag="d2")
                for h in range(nmm):
                    b, sl = divmod(g, 3)
                    g += 1
                    lhsT = xt_sb[32 * sl: 32 * sl + KF,
                                 b * 128: (b + 1) * 128]
                    rhs = yt_sb[32 * sl: 32 * sl + KF, 0:FPC]
                    nc.tensor.matmul(
                        ps[:, h * FPC: (h + 1) * FPC], lhsT, rhs,
                        start=True, stop=True,
                    )
                if u == 0:
                    nc.sync.dma_start(xy_sb[:, 1280:1920],
                                      xy_d[:, 1280:1920])
                s = sp.tile([128, 2048], BF16, tag="s")
                nc.scalar.activation(s[:, 0:w], ps[:, 0:w], AF.Sqrt,
                                     bias=epst[:])
                tmp = sp.tile([128, 2048], BF16, tag="tmp")
                nc.vector.tensor_scalar_min(tmp[:, 0:w], s[:, 0:w], CLAMP)
                nc.vector.tensor_add(acc[:, 0:w], acc[:, 0:w], tmp[:, 0:w])
                if u == len(STWIDTHS) - 2:
                    # acc cols 1024:2048 are final (the last supertile only
                    # touches 0:1024) -- ship them now to overlap the DRAM
                    # write fence with the final supertile
                    nc.sync.dma_start(out_d[:, 1024:2048],
                                      acc[:, 1024:2048])

            # tail: ship the remaining half; host does the final sum
            nc.sync.dma_start(out_d[:, 0:1024], acc[:, 0:1024])

    nc.finalize()
    return nc


_NC_CACHE = None


def _get_nc():
    global _NC_CACHE
    if _NC_CACHE is None:
        _NC_CACHE = build_nc()
    return _NC_CACHE


def _frames(c):
    o = c[1:-1]
    e1 = c[2:] - c[1:-1]
    e1 = e1 / (np.linalg.norm(e1, axis=1, keepdims=True) + EPS)
    e2 = c[:-2] - c[1:-1]
    e2 = e2 - (e2 * e1).sum(1, keepdims=True) * e1
    e2 = e2 / (np.linalg.norm(e2, axis=1, keepdims=True) + EPS)
    e3 = np.cross(e1, e2)
    R = np.stack([e1, e2, e3], 1)          # [F,3,3], rows are basis vecs
    return o, R


def make_in_maps(pred_coords, true_coords):
    pred = np.ascontiguousarray(pred_coords, dtype=np.float32)
    true = np.ascontiguousarray(true_coords, dtype=np.float32)

    # X features [N, 17]
    A = (pred * pred).sum(1) + (true * true).sum(1)
    W = (pred[:, :, None] * true[:, None, :]).reshape(N, 9)
    X = np.concatenate(
        [A[:, None], np.ones((N, 1), np.float32), pred, true, W],
        axis=1).astype(np.float32)

    # Y features [F, 17]
    po, Rp = _frames(pred)
    to, Rt = _frames(true)
    M = np.einsum('frc,frd->fcd', Rp, Rt)      # Rp^T Rt
    u = np.einsum('fcd,fd->fc', M, to)
    v = np.einsum('fcd,fc->fd', M, po)
    cf = (po * u).sum(1)
    B = (po * po).sum(1) + (to * to).sum(1)
    Y = np.concatenate(
        [np.ones((F, 1), np.float32), (B - 2 * cf + DSQ_OFF)[:, None],
         2 * (u - po), 2 * (v - to), (-2 * M).reshape(F, 9)],
        axis=1).astype(np.float32)

    # X^T layout [96, 1408]: xt[32s + k, b*128 + c] = X[(3b + s)*128 + c, k]
    # (33rd group slot unused/zero); packed at cols 512:1920 of xy
    xt = np.zeros((96, 1408), np.float32)
    Xp = np.zeros((NWIN * 3 * 128, KF), np.float32)
    Xp[:N] = X
    tmp = Xp.reshape(NWIN, 3, 128, KF)         # [b, s, c, k]
    xt.reshape(3, 32, NWIN, 128)[:, :KF] = tmp.transpose(1, 3, 0, 2)

    in_maps = []
    for i in range(NCORES):
        f0 = i * FPC
        nvalid = min(FPC, F - f0)
        Yc = np.zeros((FPC, KF), np.float32)
        Yc[:nvalid] = Y[f0: f0 + nvalid]
        xy = np.zeros((96, 1920), np.float32)
        xy[:, FPC:1920] = xt
        xy.reshape(96, -1)[:, 0:FPC].reshape(3, 32, FPC)[:, :KF] = Yc.T[None]
        in_maps.append({"xy": xy})
    return in_maps


def kernel(pred_coords, true_coords):
    nc = _get_nc()
    in_maps = make_in_maps(pred_coords, true_coords)
    res = run_bass_kernel_spmd(nc, in_maps, list(range(NCORES)))
    total = sum(float(np.asarray(r["out"], np.float32).sum())
                for r in res.results)
    return np.float32(total / (F * N) / UNIT)


# revision 27
# speedup vs baseline: 1.0267x; 1.0267x over previous
"""FAPE loss Trainium2 kernel.

Math: for frames f (built from coord triples) and points n,
  d2[f,n] = ||Rp(p_n - po_f)||^2 + ||Rt(t_n - to_f)||^2 - 2 (p_n-po_f)^T M (t_n-to_f)
with M = Rp^T Rt.  Expanding, d2[f,n] = X[n] . Y[f] with 17 features:
  X = [A_n, 1, p (3), t (3), W (9)]   A_n = ||p_n||^2 + ||t_n||^2, W = outer(p_n, t_n)
  Y = [mask, B_f - 2c_f + off, 2(u-po), 2(v-to), -2M]  u = M to, v = M^T po,
      c_f = po.u, B_f = ||po||^2 + ||to||^2
Loss = mean(min(sqrt(d2 + eps), 10)) / 10.

The O(N) feature prep (X per point, Y per frame) is done host-side in numpy
and shipped pre-transposed in the exact matmul layouts, so the device does
only the O(F*N) part: 32 fp32r matmuls (K=17), ACT sqrt, DVE clamped
accumulation, and a scalar reduce.

Sharding: frames split across 8 cores (512/core; the last core's 2 pad
frames have all-zero Y rows).  Points replicated.

Device layout per core:
  xt [96, 1408] f32r: X^T in 11 windows of 128 cols (=128 points); window
      b, slot s in {0..2} holds feature k at partition 32s+k for point
      group g = 3b + s (points g*128 ..); 33rd group slot zero.
  yt [96, 512] f32r: Y^T replicated at partition bases 0/32/64 so every
      lhsT slot finds a matching rhs.
  8 supertiles u of 4 matmuls g = 4u+h (window g//3, slot g%3) -> PSUM
      [128, 2048] f32 -> ACT sqrt(+eps) -> bf16 SBUF s.
      Finish on DVE in fast 16-bit mode: tmp = min(s, 10); acc += tmp
      (min also squashes any NaN from f32r noise on near-zero d2).
      Tail: DMA acc [128, 2048] bf16 straight out; the host sums it
      (the on-device reduce chain cost ~3us serial).
"""
import sys

for _p in ("/opt/trn_rl_repo", "/root/.axon_site/_ro/trn_rl_repo"):
    if _p not in sys.path:
        sys.path.append(_p)

import numpy as np
from concourse import bass, bacc, mybir, tile
from concourse.bass_utils import run_bass_kernel_spmd

F32 = mybir.dt.float32
F32R = mybir.dt.float32r
BF16 = mybir.dt.bfloat16
F16 = mybir.dt.float16
AF = mybir.ActivationFunctionType
OP = mybir.AluOpType

N = 4096          # points
F = N - 2         # frames (4094)
NCORES = 8
FPC = 512         # frames per core (last core: 510 real + 2 zero-pad)
KF = 17           # contraction features
EPS = 1e-8
UNIT = 10.0
CLAMP = 10.0
DSQ_OFF = 1.0     # added to every real frame's d2 so f32r noise can't push
                  # it negative (sqrt(neg)=NaN); ~3.9e-4 relative loss bias
NWIN = 11         # X^T windows of 128 points, 3 feature-slots each
STWIDTHS = [1024] + [2048] * 7 + [1024]   # tapered supertiles (32 matmuls)


def build_nc():
    nc = bacc.Bacc(None)

    # yt (cols 0:512) and xt (cols 512:1920) packed in one input tensor so
    # the sync queue issues fewer DMAs before the block-entry barrier
    xy_d = nc.dram_tensor("xy", [96, 1920], F32R, kind="ExternalInput")
    out_d = nc.dram_tensor("out", [128, 2048], BF16, kind="ExternalOutput")

    with tile.TileContext(nc) as tc:
        with (
            tc.tile_pool(name="inp", bufs=1) as inp,
            tc.tile_pool(name="sp", bufs=2) as sp,
            tc.tile_pool(name="accp", bufs=1) as accp,
            tc.tile_pool(name="psD", bufs=2, space="PSUM") as psD,
        ):
            xy_sb = inp.tile([96, 1920], F32R)
            yt_sb = xy_sb[:, 0:FPC]
            xt_sb = xy_sb[:, FPC:1920]
            # staged input DMAs on the sync HWDGE queue: the first carries
            # only what supertile 0 needs (yt + window 0), so the first
            # matmul fires as early as possible; later windows pipeline in
            # behind it
            nc.sync.dma_start(xy_sb[:, 0:640], xy_d[:, 0:640])
            nc.sync.dma_start(xy_sb[:, 640:1280], xy_d[:, 640:1280])

            epst = inp.tile([128, 1], F32)
            nc.vector.memset(epst[:], EPS)

            acc = accp.tile([128, 2048], BF16)
            nc.vector.memset(acc[:], 0.0)

            g = 0
            for u, w in enumerate(STWIDTHS):
                nmm = w // FPC
                ps = psD.tile([128, 2048], F32, tag="d2")
                for h in range(nmm):
                    b, sl = divmod(g, 3)
                    g += 1
                    lhsT = xt_sb[32 * sl: 32 * sl + KF,
                                 b * 128: (b + 1) * 128]
                    rhs = yt_sb[32 * sl: 32 * sl + KF, 0:FPC]
                    nc.tensor.matmul(
                        ps[:, h * FPC: (h + 1) * FPC], lhsT, rhs,
                        start=True, stop=True,
                    )
                if u == 0:
                    nc.sync.dma_start(xy_sb[:, 1280:1920],
                                      xy_d[:, 1280:1920])
                s = sp.tile([128, 2048], BF16, tag="s")
                nc.scalar.activation(s[:, 0:w], ps[:, 0:w], AF.Sqrt,
                                     bias=epst[:])
                tmp = sp.tile([128, 2048], BF16, tag="tmp")
                nc.vector.tensor_scalar_min(tmp[:, 0:w], s[:, 0:w], CLAMP)
                nc.vector.tensor_add(acc[:, 0:w], acc[:, 0:w], tmp[:, 0:w])
                if u == len(STWIDTHS) - 2:
                    # acc cols 1024:2048 are final (the last supertile only
                    # touches 0:1024) -- ship them now to overlap the DRAM
                    # write fence with the final supertile
                    nc.sync.dma_start(out_d[:, 1024:2048],
                                      acc[:, 1024:2048])

            # tail: ship the remaining half; host does the final sum
            nc.sync.dma_start(out_d[:, 0:1024], acc[:, 0:1024])

    nc.finalize()
    return nc


_NC_CACHE = None


def _get_nc():
    global _NC_CACHE
    if _NC_CACHE is None:
        _NC_CACHE = build_nc()
    return _NC_CACHE


def _frames(c):
    o = c[1:-1]
    e1 = c[2:] - c[1:-1]
    e1 = e1 / (np.linalg.norm(e1, axis=1, keepdims=True) + EPS)
    e2 = c[:-2] - c[1:-1]
    e2 = e2 - (e2 * e1).sum(1, keepdims=True) * e1
    e2 = e2 / (np.linalg.norm(e2, axis=1, keepdims=True) + EPS)
    e3 = np.cross(e1, e2)
    R = np.stack([e1, e2, e3], 1)          # [F,3,3], rows are basis vecs
    return o, R


def make_in_maps(pred_coords, true_coords):
    pred = np.ascontiguousarray(pred_coords, dtype=np.float32)
    true = np.ascontiguousarray(true_coords, dtype=np.float32)

    # X features [N, 17]
    A = (pred * pred).sum(1) + (true * true).sum(1)
    W = (pred[:, :, None] * true[:, None, :]).reshape(N, 9)
    X = np.concatenate(
        [A[:, None], np.ones((N, 1), np.float32), pred, true, W],
        axis=1).astype(np.float32)

    # Y features [F, 17]
    po, Rp = _frames(pred)
    to, Rt = _frames(true)
    M = np.einsum('frc,frd->fcd', Rp, Rt)      # Rp^T Rt
    u = np.einsum('fcd,fd->fc', M, to)
    v = np.einsum('fcd,fc->fd', M, po)
    cf = (po * u).sum(1)
    B = (po * po).sum(1) + (to * to).sum(1)
    Y = np.concatenate(
        [np.ones((F, 1), np.float32), (B - 2 * cf + DSQ_OFF)[:, None],
         2 * (u - po), 2 * (v - to), (-2 * M).reshape(F, 9)],
        axis=1).astype(np.float32)

    # X^T layout [96, 1408]: xt[32s + k, b*128 + c] = X[(3b + s)*128 + c, k]
    # (33rd group slot unused/zero); packed at cols 512:1920 of xy
    xt = np.zeros((96, 1408), np.float32)
    Xp = np.zeros((NWIN * 3 * 128, KF), np.float32)
    Xp[:N] = X
    tmp = Xp.reshape(NWIN, 3, 128, KF)         # [b, s, c, k]
    xt.reshape(3, 32, NWIN, 128)[:, :KF] = tmp.transpose(1, 3, 0, 2)

    in_maps = []
    for i in range(NCORES):
        f0 = i * FPC
        nvalid = min(FPC, F - f0)
        Yc = np.zeros((FPC, KF), np.float32)
        Yc[:nvalid] = Y[f0: f0 + nvalid]
        xy = np.zeros((96, 1920), np.float32)
        xy[:, FPC:1920] = xt
        xy.reshape(96, -1)[:, 0:FPC].reshape(3, 32, FPC)[:, :KF] = Yc.T[None]
        in_maps.append({"xy": xy})
    return in_maps


def kernel(pred_coords, true_coords):
    nc = _get_nc()
    in_maps = make_in_maps(pred_coords, true_coords)
    res = run_bass_kernel_spmd(nc, in_maps, list(range(NCORES)))
    total = sum(float(np.asarray(r["out"], np.float32).sum())
                for r in res.results)
    return np.float32(total / (F * N) / UNIT)
